# revision 24
# baseline (speedup 1.0000x reference)
"""Trainium2 Bass kernel for nn_GAT_78546361909763.

Computes, per sample b (B=16, N=2048, D=256):
    s_j = x @ w[:D];  s_i = x @ w[D:]
    att[i,j] = s_i[i] + s_j[j]
    att = LayerNorm_{(N,N)}(att) * gamma + beta    (gamma==1, beta==0 fast path)
    att = LeakyReLU_{0.2}(att)
    att = softmax(att, axis=-2)                     (normalize each column j over i)
    out = sigmoid(att @ x)

Key algebraic facts exploited on-device:
  * LayerNorm stats over the (N,N) matrix decompose: mean = mean(s_i)+mean(s_j),
    var = var(s_i)+var(s_j), so stats come from the two (N,) vectors.
  * exp(leaky(z)) with z = r*(s+c) equals exp(r*(max(s, 0.2*s - 0.8*c) + c)),
    i.e. one DVE tensor_scalar + one DVE max + one ACT Exp (with per-partition
    bias r*c and scale r) per tile.
  * The softmax denominator depends only on the contraction index j, so it
    folds into x:  out[i,d] = sum_j expT[j,i] * (x[j,d] / den[j]).
  * softmax is shift-invariant: a global -2 shift inside exp keeps the fp8
    et values in [0, ~30] so float8e4 (max 240) holds them losslessly enough.
  * sigmoid(y) = 0.5 + 0.5*tanh(y/2); Tanh and Exp share one ACT table set.

Layout: att is built transposed (j on partitions, i on the free axis) so the
softmax reduction is a free-axis accumulation (free via ACT accum_out) and the
final matmul out_T[d,i] = sum_j xt[j,d] * expT[j,i] contracts j on partitions.
et and xt are written as float8e4 pair-tiles (two adjacent j-chunks) and the
matmul runs in DoubleRow perf mode (two 128-deep k-tiles per instruction).
The kernel emits out_T (B, D, N); the host transposes back.

Sharding: data-parallel over B across 8 cores (2 samples per core).
"""

import sys

sys.path.insert(0, "/opt/trn_rl_repo")

import numpy as np

import concourse.bass as bass
import concourse.tile as tile
from concourse import bacc, bass_isa, mybir
from concourse.bass_utils import run_bass_kernel_spmd

B, N, D = 16, 2048, 256
NCORES = 8
BL = B // NCORES            # samples per core
NCH = N // 128              # 16 row chunks of 128
XG = 4                      # x chunks per DMA group
NEG = 0.2                   # leaky relu slope
EPS = 1e-14
ESHIFT = 2.0                # global exp shift (cancels in softmax)
XTS = 256.0                 # xt fp8 scale (compensated in the tanh scale)
FP = mybir.dt.float32
BF = mybir.dt.bfloat16
F8 = mybir.dt.float8e4
AF = mybir.ActivationFunctionType
ALU = mybir.AluOpType
DR = mybir.MatmulPerfMode.DoubleRow


def _emit_rsqrt(nc, pool, v_ap):
    """r = 1/sqrt(v + EPS) on DVE only (avoids ACT table switches).

    Fast inverse sqrt seed + 3 Newton iterations on a [128,1] f32 tile.
    """
    vv = pool.tile([128, 1], FP, tag="nwt_vv")
    nc.vector.tensor_scalar(vv[:, :], v_ap, float(EPS), None, ALU.add)
    # seed: y0 = bitcast(0x5f3759df - (bitcast(vv) >> 1))
    yi = pool.tile([128, 1], mybir.dt.int32, tag="nwt_yi")
    nc.vector.tensor_scalar(yi[:, :], vv[:, :].bitcast(mybir.dt.int32), 1, None,
                            ALU.arith_shift_right)
    # y0i = MAGIC - (vi >> 1), as (-1)*(vi>>1) + MAGIC (arith-only ops)
    nc.vector.tensor_scalar(yi[:, :], yi[:, :], -1, 0x5F3759DF,
                            ALU.mult, ALU.add)
    y = pool.tile([128, 1], FP, tag="nwt_y")
    nc.vector.tensor_copy(y[:, :], yi[:, :].bitcast(FP))
    t = pool.tile([128, 1], FP, tag="nwt_t")
    for _ in range(2):
        nc.vector.tensor_tensor(t[:, :], y[:, :], y[:, :], ALU.mult)
        nc.vector.tensor_tensor(t[:, :], t[:, :], vv[:, :], ALU.mult)
        nc.vector.tensor_scalar(t[:, :], t[:, :], -0.5, 1.5, ALU.mult, ALU.add)
        nc.vector.tensor_tensor(y[:, :], y[:, :], t[:, :], ALU.mult)
    return y


def _emit_kernel(tc, out_d, x_d, w_d, reps=1):
    # python-unrolled reps (used only for timing amplification; a For_i
    # device loop wedges the exec unit on this runtime)
    for _ in range(reps):
        _emit_body(tc, out_d, x_d, w_d)


def _emit_body(tc, out_d, x_d, w_d):
    nc = tc.nc
    ctxs = []

    def mkpool(name, bufs, **kw):
        p = tc.alloc_tile_pool(name=name, bufs=bufs, **kw)
        ctxs.append(p)
        return p

    consts = mkpool("consts", 1)
    px = mkpool("px", 2 * NCH // XG + 2)  # x chunk groups, f32 [128, XG*256]
    pscr = mkpool("pscr", 2)         # matvec product scratch
    psmall = mkpool("psmall", 2)     # per-sample small tiles
    pnwt = mkpool("pnwt", 2)         # newton temps
    prepl = mkpool("prepl", 2)       # s_repl / s02_repl
    prow = mkpool("prow", 2)         # [1, N] gather row
    pv0 = mkpool("pv0", 10)          # build tiles bf16 [128, N]
    pexp = mkpool("pexp", 8)         # exp pair tiles fp8 [128, 2N]
    pxt = mkpool("pxt", 8)           # x~ pair tiles fp8 [128, 2D]
    pstg = mkpool("pstg", 3)         # output staging f32 [128, N]
    ppsum = mkpool("ppsum", 2, space="PSUM")
    pdram = mkpool("pdram", 2, space="DRAM")

    zero = consts.tile([128, 1], FP)
    nc.vector.memset(zero[:, :], 0.0)

    xmap = {}         # (s, c) -> (tile, col offset)

    def xch(s, c):
        t, off = xmap[(s, c)]
        return t[:, off * D:(off + 1) * D]

    def emit_xload(s, c0, nch):
        xt_ = px.tile([128, XG * D], FP, tag="xgrp", name=f"x_{s}_{c0}")
        src = x_d[s, :, :].rearrange("(g p) d -> p g d", p=128)
        nc.sync.dma_start(
            xt_[:, :nch * D].rearrange("p (g d) -> p g d", g=nch),
            src[:, c0:c0 + nch, :])
        for k in range(nch):
            xmap[(s, c0 + k)] = (xt_, k)

    def emit_matvec(s, c, stats_in, h):
        scr = pscr.tile([128, D], FP, tag="scr", name=f"scr_{s}_{c}_{h}")
        nc.vector.scalar_tensor_tensor(
            scr[:, :], xch(s, c), 0.0,
            w_sb[:, h * D:(h + 1) * D],
            ALU.bypass, ALU.mult,
            accum_out=stats_in[:, h * NCH + c:h * NCH + c + 1],
        )

    def emit_row_path(s, stats_in):
        # s_i columns -> row -> broadcast; only needs the h=1 matvec accums,
        # so it runs while the h=0 pass is still going.
        si_bf = psmall.tile([128, 32], BF, tag="si_bf", name=f"si_bf_{s}")
        nc.vector.memset(si_bf[:, NCH:], 0.0)
        nc.vector.tensor_copy(si_bf[:, 0:NCH], stats_in[:, NCH:2 * NCH])
        rowt = psmall.tile([32, 128], BF, tag="rowt", name=f"rowt_{s}")
        for b in range(4):
            nc.vector.transpose(rowt[0:32, b * 32:(b + 1) * 32],
                                si_bf[b * 32:(b + 1) * 32, :])
        dlin = pdram.tile([NCH, 128], BF, tag="dlin", name=f"dlin_{s}")
        nc.sync.dma_start(dlin[:, :], rowt[0:NCH, :])
        # replicate the row into all partitions in one DMA (0-stride source)
        s_repl = prepl.tile([128, N], BF, tag="s_repl", name=f"s_repl_{s}")
        nc.sync.dma_start(
            s_repl[:, :],
            dlin[:, :].rearrange("a b -> () (a b)").partition_broadcast(128))
        return s_repl

    def emit_stats_math(s, stats_in, s_repl):
        nc.vector.tensor_tensor(stats_in[:, 2 * NCH:], stats_in[:, :2 * NCH],
                                stats_in[:, :2 * NCH], ALU.mult)
        sums4 = psmall.tile([128, 4], FP, tag="sums4", name=f"sums4_{s}")
        nc.vector.tensor_reduce(
            sums4[:, :],
            stats_in[:, :].rearrange("p (g c) -> p g c", g=4),
            mybir.AxisListType.X, ALU.add)
        tot4 = psmall.tile([128, 4], FP, tag="tot4", name=f"tot4_{s}")
        nc.gpsimd.partition_all_reduce(tot4[:, :], sums4[:, :], 128,
                                       bass_isa.ReduceOp.add)
        mean4 = psmall.tile([128, 4], FP, tag="mean4", name=f"mean4_{s}")
        nc.vector.tensor_scalar(mean4[:, :], tot4[:, :], 1.0 / N, None, ALU.mult)
        m = psmall.tile([128, 1], FP, tag="m", name=f"m_{s}")
        nc.vector.tensor_tensor(m[:, :], mean4[:, 0:1], mean4[:, 1:2], ALU.add)
        msq = psmall.tile([128, 2], FP, tag="msq", name=f"msq_{s}")
        nc.vector.tensor_tensor(msq[:, :], mean4[:, 0:2], mean4[:, 0:2], ALU.mult)
        q = psmall.tile([128, 1], FP, tag="q", name=f"q_{s}")
        nc.vector.tensor_tensor(q[:, :], mean4[:, 2:3], mean4[:, 3:4], ALU.add)
        m2 = psmall.tile([128, 1], FP, tag="m2", name=f"m2_{s}")
        nc.vector.tensor_tensor(m2[:, :], msq[:, 0:1], msq[:, 1:2], ALU.add)
        v = psmall.tile([128, 1], FP, tag="v", name=f"v_{s}")
        nc.vector.tensor_tensor(v[:, :], q[:, :], m2[:, :], ALU.subtract)
        r = _emit_rsqrt(nc, pnwt, v[:, :])
        cc = psmall.tile([128, NCH], FP, tag="cc", name=f"cc_{s}")
        nc.vector.tensor_scalar(cc[:, :], stats_in[:, 0:NCH], m[:, 0:1], None,
                                ALU.subtract)
        nb08 = psmall.tile([128, NCH], FP, tag="nb08", name=f"nb08_{s}")
        nc.vector.tensor_scalar(nb08[:, :], cc[:, :], -(1.0 - NEG), None, ALU.mult)
        # rc = r*c - ESHIFT (the shift cancels in softmax, keeps fp8 in range)
        rc = psmall.tile([128, NCH], FP, tag="rc", name=f"rc_{s}")
        nc.vector.tensor_scalar(rc[:, :], cc[:, :], r[:, 0:1], -ESHIFT,
                                ALU.mult, ALU.add)
        return dict(r=r, rc=rc, nb08=nb08, s_repl=s_repl)

    state = {}

    def new_sctx(s):
        return dict(
            po=[ppsum.tile([128, N], FP, tag="po", name=f"po_{s}_{d}")
                for d in range(2)],
            den=psmall.tile([128, NCH], FP, tag="den", name=f"den_{s}"),
            dinv=psmall.tile([128, NCH], FP, tag="dinv", name=f"dinv_{s}"),
            stv=state[s], etp=None, mmq=[])

    def emit_mm(s, sc, etp, xtp, c):
        lhs3 = xtp[:, :].rearrange("p (k d) -> p k d", k=2)
        rhs3 = etp[:, :].rearrange("p (k n) -> p k n", k=2)
        for d in range(2):
            for nn in range(4):
                nc.tensor.matmul(
                    sc["po"][d][:, nn * 512:(nn + 1) * 512],
                    lhs3[:, :, d * 128:(d + 1) * 128],
                    rhs3[:, :, nn * 512:(nn + 1) * 512],
                    start=(c == 1), stop=(c == NCH - 1),
                    perf_mode=DR)

    def emit_build(s, c, sc, defer_mm=False):
        stv, den, dinv = sc["stv"], sc["den"], sc["dinv"]
        if c % 2 == 0:
            sc["etp"] = pexp.tile([128, 2 * N], F8, tag="exp",
                                  name=f"etp_{s}_{c}")
        etp = sc["etp"]
        v0a = pv0.tile([128, N], BF, tag="v0a", name=f"v0a_{s}_{c}")
        nc.vector.tensor_scalar(v0a[:, :], stv["s_repl"][:, :],
                                NEG, stv["nb08"][:, c:c + 1],
                                ALU.mult, ALU.add)
        v0 = pv0.tile([128, N], BF, tag="v0", name=f"v0_{s}_{c}")
        nc.vector.tensor_tensor(v0[:, :], v0a[:, :], stv["s_repl"][:, :],
                                ALU.max)
        nc.scalar.activation(
            etp[:, (c % 2) * N:(c % 2 + 1) * N], v0[:, :], AF.Exp,
            bias=stv["rc"][:, c:c + 1], scale=stv["r"][:, 0:1],
            accum_out=den[:, c:c + 1])
        if c % 2 == 1:
            nc.vector.reciprocal(dinv[:, c - 1:c + 1], den[:, c - 1:c + 1])
            xtp = pxt.tile([128, 2 * D], F8, tag="xt", name=f"xtp_{s}_{c}")
            for k, cc_ in enumerate((c - 1, c)):
                nc.gpsimd.tensor_scalar(xtp[:, k * D:(k + 1) * D],
                                        xch(s, cc_), dinv[:, cc_:cc_ + 1],
                                        XTS, ALU.mult, ALU.mult)
            if defer_mm:
                sc["mmq"].append((etp, xtp, c))
            else:
                emit_mm(s, sc, etp, xtp, c)

    def emit_drain(s, sc, last):
        # sigmoid(y) = 0.5 + 0.5*tanh(y/2), pipelined in pieces; the last
        # sample uses finer pieces to shorten the exposed tail, the others
        # coarser ones to save per-op ACT overhead
        np_ = 4 if last else 2
        for d in range(2):
            stg = pstg.tile([128, N], FP, tag="stg", name=f"stg_{s}_{d}")
            for q_ in range(np_):
                sl = slice(q_ * (N // np_), (q_ + 1) * (N // np_))
                nc.scalar.activation(stg[:, sl], sc["po"][d][:, sl], AF.Tanh,
                                     bias=zero[:, 0:1], scale=0.5 / XTS)
                # tail: DVE is idle by then and faster than Pool
                eng = nc.vector if last else nc.gpsimd
                eng.tensor_scalar(stg[:, sl], stg[:, sl], 0.5, 0.5,
                                  ALU.mult, ALU.add)
                nc.sync.dma_start(out_d[s, d * 128:(d + 1) * 128, sl],
                                  stg[:, sl])

    # ---- startup: sample 0 prologue ----
    # chunk-0 x DMA first so the first matvec starts ~1.2us in; w next.
    assert BL == 2
    st_in = {0: psmall.tile([128, 4 * NCH], FP, tag="stats_in", name="si0")}
    w_sb = consts.tile([128, 2 * D], FP)
    nc.sync.dma_start(w_sb[:, :], w_d[:, :])
    emit_xload(0, 0, 1)
    emit_xload(0, 1, 1)
    emit_xload(0, 2, 2)
    for c0 in range(4, NCH, XG):
        emit_xload(0, c0, XG)
    for c in range(NCH):
        emit_matvec(0, c, st_in[0], h=0)
        emit_matvec(0, c, st_in[0], h=1)
    srepl0 = emit_row_path(0, st_in[0])
    state[0] = emit_stats_math(0, st_in[0], srepl0)
    sc0 = new_sctx(0)
    st_in[1] = psmall.tile([128, 4 * NCH], FP, tag="stats_in", name="si1")
    for c0 in range(0, NCH, XG):
        emit_xload(1, c0, XG)
    # s1 matvecs fill the DVE idle window while s0's row DMA round-trip and
    # broadcast complete (the first build waits on s_repl anyway)
    for c in range(4):
        emit_matvec(1, c, st_in[1], h=0)
        emit_matvec(1, c, st_in[1], h=1)

    # ---- sample 0 chunks; sample 1 prologue rides along ----
    srepl1 = None
    for c in range(10):
        emit_build(0, c, sc0)
        if c < 6:
            for cc_ in (2 * c + 4, 2 * c + 5):
                emit_matvec(1, cc_, st_in[1], h=0)
                emit_matvec(1, cc_, st_in[1], h=1)
        elif c == 6:
            srepl1 = emit_row_path(1, st_in[1])
            state[1] = emit_stats_math(1, st_in[1], srepl1)
    sc1 = new_sctx(1)
    # interleave s1's first chunks into s0's tail to keep ACT packed across
    # the sample boundary; their matmuls are deferred so the PE queue keeps
    # all s0 matmuls (and the PSUM handoff) ahead of s1's.
    k1 = 0
    for c in range(10, NCH):
        emit_build(0, c, sc0)
        emit_build(1, k1, sc1, defer_mm=True)
        k1 += 1
        emit_build(1, k1, sc1, defer_mm=True)
        k1 += 1
    for (etp_, xtp_, c_) in sc1["mmq"]:
        emit_mm(1, sc1, etp_, xtp_, c_)
    emit_drain(0, sc0, last=False)
    while k1 < NCH:
        emit_build(1, k1, sc1)
        k1 += 1
    emit_drain(1, sc1, last=True)

    for p in reversed(ctxs):
        p.release()


_NC = {}


def _get_nc(reps=1):
    if reps not in _NC:
        nc = bacc.Bacc("TRN2", target_bir_lowering=False, debug=False,
                       enable_asserts=False, num_devices=NCORES)
        x_d = nc.dram_tensor("x", [BL, N, D], FP, kind="ExternalInput").ap()
        w_d = nc.dram_tensor("w", [128, 2 * D], FP, kind="ExternalInput").ap()
        out_d = nc.dram_tensor("out_t", [BL, D, N], FP, kind="ExternalOutput").ap()
        with tile.TileContext(nc) as tc:
            _emit_kernel(tc, out_d, x_d, w_d, reps=reps)
        nc.compile()
        _NC[reps] = nc
    return _NC[reps]


def _numpy_fallback(x, weight, gamma, beta):
    out = np.empty((x.shape[0], x.shape[1], x.shape[2]), np.float32)
    d = x.shape[-1]
    for b in range(x.shape[0]):
        xb = x[b].astype(np.float64)
        s_j = xb @ weight[:d].astype(np.float64)
        s_i = xb @ weight[d:].astype(np.float64)
        att = s_i[:, None] + s_j[None, :]
        mean = att.mean()
        var = ((att - mean) ** 2).mean()
        att = (att - mean) / np.sqrt(var + EPS) * gamma + beta
        att = np.where(att >= 0, att, NEG * att)
        att = att - att.max(axis=0, keepdims=True)
        e = np.exp(att)
        att = e / e.sum(axis=0, keepdims=True)
        out[b] = 1.0 / (1.0 + np.exp(-(att @ xb)))
    return out


def run(inputs, trace=False):
    """Run the device kernel. Returns (output, exec_time_ns or None)."""
    x = np.ascontiguousarray(np.asarray(inputs["x"], dtype=np.float32))
    w = np.asarray(inputs["weight"], dtype=np.float32)
    w_repl = np.ascontiguousarray(np.broadcast_to(w, (128, 2 * D)))
    nc = _get_nc()
    in_maps = [
        {"x": np.ascontiguousarray(x[i * BL:(i + 1) * BL]), "w": w_repl}
        for i in range(NCORES)
    ]
    try:
        res = run_bass_kernel_spmd(nc, in_maps, core_ids=list(range(NCORES)),
                                   trace=trace)
    except ModuleNotFoundError:
        res = run_bass_kernel_spmd(nc, in_maps, core_ids=list(range(NCORES)),
                                   trace=False)
    parts = [np.transpose(res.results[i]["out_t"], (0, 2, 1))
             for i in range(NCORES)]
    out = np.concatenate(parts, axis=0)
    return out, res.exec_time_ns


def kernel(**inputs):
    gamma = np.asarray(inputs["gamma"])
    beta = np.asarray(inputs["beta"])
    if not (np.all(gamma == 1.0) and np.all(beta == 0.0)):
        return _numpy_fallback(
            np.asarray(inputs["x"], np.float32),
            np.asarray(inputs["weight"], np.float32),
            gamma.astype(np.float32), beta.astype(np.float32))
    out, _ = run(inputs)
    return out
